# revision 27
# baseline (speedup 1.0000x reference)
"""Expert-parallel MoE FFN kernel for Trainium2 (8 NeuronCores).

Reference computation (per expert e):
    y[:, e*C:(e+1)*C, :] = gelu(x_e @ w1[e] + b1[e]) @ w2[e] + b2[e]

Sharding: expert-parallel — core e owns expert e (E == n_cores == 8) and the
matching chunk of dim 1 of `inputs`. No cross-core communication.

Per-core dataflow (T=16384 tokens, D=512, F=2048), all matmuls bf16:
  - X tiles load token-major [128t, 512d] fp32, are cast to bf16 (DVE) and
    transposed to [128d, 4, 512t] via a DRAM bounce + XBAR DMA-transpose
    (keeps the TensorE stream pure matmuls so the HAM clock stays warm).
  - mm1: H^T[f, t] += W1[d, f].T @ X^T[d, t]; gelu+b1 fused on ScalarE
    (f on partitions -> b1 is a per-partition bias), H stored bf16.
  - mm2: Y[t, d] += (H^T[f, t128]).T @ W2[f, d] with H^T as the stationary
    operand, so Y comes out token-major and stores contiguously.
"""

import numpy as np
import ml_dtypes

import concourse.bacc as bacc
import concourse.bass as bass
import concourse.mybir as mybir
import concourse.tile as tile
from concourse.bass_utils import run_bass_kernel_spmd
from concourse.masks import make_identity

B, EC, D = 16, 8192, 512
E, F = 8, 2048
C = EC // E            # capacity per expert = 1024
T = B * C              # tokens per expert/core = 16384
P = 128
DSUB = D // P          # 4
FSUB = F // P          # 16
TCHUNK = 512
TS = TCHUNK // P       # 4
N_CORES = 8

# Stash of the last BassKernelResults (for test harness profiling).
LAST_RESULT = None


def build_nc(n_tokens: int = T, act_func=None):
    if act_func is None:
        act_func = mybir.ActivationFunctionType.Gelu_apprx_tanh
    nchunk = n_tokens // TCHUNK
    nc = bacc.Bacc(
        "TRN2",
        target_bir_lowering=False,
        debug=False,
        num_devices=N_CORES,
    )
    x = nc.dram_tensor("x", [n_tokens, D], mybir.dt.float32, kind="ExternalInput").ap()
    w1 = nc.dram_tensor("w1", [P, DSUB, F], mybir.dt.bfloat16, kind="ExternalInput").ap()
    b1 = nc.dram_tensor("b1", [P, FSUB], mybir.dt.float32, kind="ExternalInput").ap()
    w2 = nc.dram_tensor("w2", [P, FSUB, D], mybir.dt.bfloat16, kind="ExternalInput").ap()
    b2 = nc.dram_tensor("b2", [P, D], mybir.dt.float32, kind="ExternalInput").ap()
    y = nc.dram_tensor("y", [n_tokens, D], mybir.dt.float32, kind="ExternalOutput").ap()

    with tile.TileContext(nc) as tc:
        with (
            tc.tile_pool(name="consts", bufs=1) as consts,
            tc.tile_pool(name="xin", bufs=4) as xin_pool,
            tc.tile_pool(name="xbf", bufs=3) as xbf_pool,
            tc.tile_pool(name="xt", bufs=6) as xt_pool,
            tc.tile_pool(name="h", bufs=2) as h_pool,
            tc.tile_pool(name="yout", bufs=4) as y_pool,
            tc.tile_pool(name="xdram", bufs=4, space="DRAM") as xdram_pool,
            tc.tile_pool(name="ps_t", bufs=2, space="PSUM") as ps_t,
            tc.tile_pool(name="ps_h", bufs=4, space="PSUM") as ps_h,
            tc.tile_pool(name="ps_y", bufs=2, space="PSUM") as ps_y,
        ):
            ident = consts.tile([P, P], mybir.dt.bfloat16)
            make_identity(nc, ident)
            # Queue roles (keeps each sequencer's waits harmless):
            #   gpsimd/SWDGE: bulk x-in, y-out, weights (parallel desc-gen)
            #   scalar/HWDGE: bounce stores (wait only on DVE cast) + gelu
            #   sync/HWDGE:   XBAR transposes only (long waits, block nothing)
            def load_x(c):
                t = xin_pool.tile([P, TS, D], mybir.dt.float32, tag="x_nat")
                nc.gpsimd.dma_start(
                    t,
                    x[c * TCHUNK:(c + 1) * TCHUNK, :].rearrange(
                        "(ts p) d -> p ts d", p=P
                    ),
                )
                return t

            # Interleave the first x-chunk prefetches with the weight loads
            # on the gpsimd queue, in deadline order: w1 is needed ~16us,
            # x2/x3 feed the transpose pipeline, w2 ~35us, b2 ~45us.
            # Whole-tensor weight DMAs are contiguous per partition
            # (128 x 16 KiB descriptors) — splitting them makes 1 KiB
            # descriptors and clogs the queue.
            x_pre = {c: load_x(c) for c in range(min(2, nchunk))}
            w1_sb = consts.tile([P, DSUB, F], mybir.dt.bfloat16)
            nc.gpsimd.dma_start(w1_sb, w1)
            b1_sb = consts.tile([P, FSUB], mybir.dt.float32)
            nc.gpsimd.dma_start(b1_sb, b1)
            for c in range(2, min(4, nchunk)):
                x_pre[c] = load_x(c)

            # Chunks 0..PE_CHUNKS-1 are transposed on the TensorE (PE is idle
            # during startup and the bounce+XBAR chain is ~30us deep). Later
            # chunks use the DMA path, with the whole cast->bounce->XBAR
            # chain EMITTED `LOOKAHEAD` chunks early so each in-order engine
            # stream runs it well ahead of its consumer.
            PE_CHUNKS = 1
            LOOKAHEAD = 3
            xt_tiles = {}

            def emit_cast(c):
                x_nat = x_pre.pop(c) if c in x_pre else load_x(c)
                x_bf = xbf_pool.tile([P, TS, D], mybir.dt.bfloat16, tag="x_bf")
                nc.vector.tensor_copy(x_bf, x_nat)
                return x_bf

            def emit_dma_chain(c):
                # Transpose via DRAM bounce + XBAR: bounce is written
                # partition-major (fully contiguous); the XBAR then
                # transposes one 128-token strip at a time
                # ([128, 512] -> [512, 128]) with a 4 KiB row stride.
                x_bf = emit_cast(c)
                x_dram = xdram_pool.tile([P, TS, D], mybir.dt.bfloat16)
                nc.scalar.dma_start(x_dram, x_bf)
                xt = xt_pool.tile([P, DSUB, TCHUNK], mybir.dt.bfloat16, tag="xt")
                for ts in range(TS):
                    nc.sync.dma_start_transpose(
                        xt[:, :, ts * P:(ts + 1) * P], x_dram[:, ts, :]
                    )
                xt_tiles[c] = xt

            def emit_pe_chain(c):
                x_bf = emit_cast(c)
                xt = xt_pool.tile([P, DSUB, TCHUNK], mybir.dt.bfloat16, tag="xt")
                for ts in range(TS):
                    for ds in range(DSUB):
                        pst = ps_t.tile([P, P], mybir.dt.bfloat16)
                        nc.tensor.transpose(
                            pst, x_bf[:, ts, ds * P:(ds + 1) * P], ident
                        )
                        nc.vector.tensor_copy(
                            xt[:, ds, ts * P:(ts + 1) * P], pst
                        )
                xt_tiles[c] = xt

            # Pre-emit: chunk 0 on the PE, chunks 1..3 via the DMA path, so
            # every engine's in-order stream starts with the transpose
            # pipeline (casts on DVE, bounces on ACT, XBARs on sync) before
            # any compute-dependent work can block it. Then w2/b2 (needed
            # ~35us) follow the early x loads on the gpsimd queue.
            emit_pe_chain(0)
            for c in range(1, min(1 + LOOKAHEAD, nchunk)):
                emit_dma_chain(c)
            w2_sb = consts.tile([P, FSUB, D], mybir.dt.bfloat16)
            nc.gpsimd.dma_start(w2_sb, w2)
            b2_sb = consts.tile([P, D], mybir.dt.float32)
            nc.gpsimd.dma_start(b2_sb, b2)

            for c in range(nchunk):
                la = c + LOOKAHEAD
                if la < nchunk and la not in xt_tiles:
                    emit_dma_chain(la)
                if c not in xt_tiles:
                    emit_dma_chain(c)
                xt = xt_tiles.pop(c)

                # mm1 + fused gelu/bias: H^T[f, t] bf16.
                h = h_pool.tile([P, FSUB, TCHUNK], mybir.dt.bfloat16)
                for fs in range(FSUB):
                    ph = ps_h.tile([P, TCHUNK], mybir.dt.float32)
                    for ds in range(DSUB):
                        nc.tensor.matmul(
                            ph,
                            lhsT=w1_sb[:, ds, fs * P:(fs + 1) * P],
                            rhs=xt[:, ds, :],
                            start=(ds == 0),
                            stop=(ds == DSUB - 1),
                        )
                    nc.scalar.activation(
                        h[:, fs, :],
                        ph,
                        act_func,
                        bias=b1_sb[:, fs:fs + 1],
                        scale=1.0,
                    )

                # mm2: Y[t, d] per 128-token subtile; + b2; store.
                for ts in range(TS):
                    py = ps_y.tile([P, D], mybir.dt.float32)
                    for fs in range(FSUB):
                        nc.tensor.matmul(
                            py,
                            lhsT=h[:, fs, ts * P:(ts + 1) * P],
                            rhs=w2_sb[:, fs, :],
                            start=(fs == 0),
                            stop=(fs == FSUB - 1),
                        )
                    y_sb = y_pool.tile([P, D], mybir.dt.float32)
                    nc.vector.tensor_add(y_sb, py, b2_sb)
                    r0 = c * TCHUNK + ts * P
                    nc.gpsimd.dma_start(y[r0:r0 + P, :], y_sb)

    nc.compile()
    return nc


_NC_CACHE = {}


def _get_nc(n_tokens: int = T):
    if n_tokens not in _NC_CACHE:
        _NC_CACHE[n_tokens] = build_nc(n_tokens)
    return _NC_CACHE[n_tokens]


def make_in_maps(inputs, w1, b1, w2, b2):
    """Shard + lay out host-side: core e gets expert e."""
    bf16 = ml_dtypes.bfloat16
    inputs = np.asarray(inputs)
    w1, b1 = np.asarray(w1), np.asarray(b1)
    w2, b2 = np.asarray(w2), np.asarray(b2)
    in_maps = []
    for e in range(E):
        x_e = np.ascontiguousarray(
            inputs[:, e * C:(e + 1) * C, :], dtype=np.float32
        ).reshape(T, D)
        # w1[e] [D, F] -> [P, DSUB, F] with d = ds*128 + p
        w1_e = np.ascontiguousarray(
            w1[e].reshape(DSUB, P, F).transpose(1, 0, 2).astype(bf16)
        )
        # b1[e] [F] -> [P, FSUB] with f = fs*128 + p
        b1_e = np.ascontiguousarray(
            b1[e].reshape(FSUB, P).T.astype(np.float32)
        )
        # w2[e] [F, D] -> [P, FSUB, D] with f = fs*128 + p
        w2_e = np.ascontiguousarray(
            w2[e].reshape(FSUB, P, D).transpose(1, 0, 2).astype(bf16)
        )
        # b2[e] [D] -> broadcast to [P, D]
        b2_e = np.ascontiguousarray(
            np.broadcast_to(b2[e].astype(np.float32), (P, D))
        )
        in_maps.append(
            {"x": x_e, "w1": w1_e, "b1": b1_e, "w2": w2_e, "b2": b2_e}
        )
    return in_maps


def kernel(inputs, w1, b1, w2, b2):
    global LAST_RESULT
    nc = _get_nc(T)
    in_maps = make_in_maps(inputs, w1, b1, w2, b2)
    res = run_bass_kernel_spmd(nc, in_maps, core_ids=list(range(N_CORES)))
    LAST_RESULT = res
    out = np.empty((B, EC, D), dtype=np.float32)
    for e in range(E):
        out[:, e * C:(e + 1) * C, :] = res.results[e]["y"].reshape(B, C, D)
    return out
